# revision 1
# baseline (speedup 1.0000x reference)
"""Trainium2 Bass kernel for nn_ConvolutionalSelfAttention.

The reference network collapses algebraically. Per image b (Xt = batch[b]
viewed [C, HW], c-major):
  K_t = key_w @ Xt + key_b          [C, HW]
  Q_t = query_w @ Xt + query_b      [C, HW]
  v   = value_w @ Xt + value_b      [HW]
  rk[n] = 1/||K_t[:,n]||, rq[m] = 1/||Q_t[:,m]||
  E[n,m] = exp(rk[n] rq[m] (K_t[:,n] . Q_t[:,m]))       (full 1024x1024 Gram)
  V[m] = (sum_n v[n] E[n,m]) / (sum_n E[n,m])
  P[c,m] = Xt[c,m] V[m]
  out[b,c,i,j] = 3x3 valid box-sum of P over the spatial grid

Sharding: data-parallel over batch (16 images over 8 cores, 2 each).
Matmuls in float32r (full PE speed, ~1.5e-4 rounding).
Row-blocks of E use the permuted order n = p*8 + j so every partition-
transpose bounce through DRAM moves 32B-contiguous lines per partition.
"""
import os
import numpy as np

os.environ.setdefault("BASS_NEVER_TRACE", "1")

import contextlib

import concourse.bass as bass
import concourse.bacc as bacc
import concourse.tile as tile
from concourse import mybir
from concourse.bass_utils import run_bass_kernel_spmd

F32 = mybir.dt.float32
F32R = mybir.dt.float32r
BF16 = mybir.dt.bfloat16
AF = mybir.ActivationFunctionType

B, C, H, W = 16, 256, 32, 32
HW = H * W            # 1024
CH = CW = 30
NF = CH * CW          # 900
NCORES = 8
BL = B // NCORES      # images per core
NCH = C // 128        # channel chunks
NNJ = HW // 128       # position chunks


def _ap(t, extra_off, pattern):
    return bass.AP(tensor=t.tensor, offset=t.offset + extra_off,
                   ap=[list(x) for x in pattern])


def build_program():
    nc = bacc.Bacc("TRN2", target_bir_lowering=False, debug=False,
                   num_devices=NCORES)
    # walrus's lower_act places activation-table loads; bacc's pre-placed
    # loads produce NEFFs this runtime refuses to load.
    nc.insert_act_table_loads = lambda: None

    def din(name, shape, dt):
        return nc.dram_tensor(name, list(shape), dt, kind="ExternalInput").ap()

    x_d = din("x", (BL, C, HW), F32R)
    wall_d = din("wall", (C, 2 * C + 2), F32R)   # [key_w.T | query_w.T | value_w.T]
    ball_d = din("ball", (128, 2 * NCH), F32)    # [bk | bq]
    bv_d = din("bv", (1, 2), F32)
    id_d = din("ident", (128, 128), F32R)

    out_d = nc.dram_tensor("out", [BL, C, NF], F32, kind="ExternalOutput").ap()

    with tile.TileContext(nc) as tc:
        with contextlib.ExitStack() as ctx:
            consts = ctx.enter_context(tc.tile_pool(name="consts", bufs=1))
            sbuf = ctx.enter_context(tc.tile_pool(name="sbuf", bufs=2))
            epool = ctx.enter_context(tc.tile_pool(name="epool", bufs=4))
            big = ctx.enter_context(tc.tile_pool(name="big", bufs=3, space="PSUM"))
            small = ctx.enter_context(tc.tile_pool(name="small", bufs=1, space="PSUM"))
            dramp = ctx.enter_context(tc.tile_pool(name="dramp", bufs=2, space="DRAM"))

            # ---------------- constants ----------------
            # critical-path-first load order: wq half + image-0 x pieces lead
            # the two dispatch queues (sync, scalar); identity loads last.
            wall_t = consts.tile([128, NCH, 2 * C + 2], F32R, tag="wall", name="wall")
            wallv = wall_d.rearrange("(kc p) m -> p kc m", p=128)
            nc.sync.dma_start(out=wall_t[:, :, 0:C], in_=wallv[:, :, 0:C])
            wq_t = wall_t[:, :, 0:C]
            wk_t = wall_t[:, :, C:2 * C]
            wv_t = wall_t[:, :, 2 * C:2 * C + 2]
            ball_t = consts.tile([128, 2 * NCH], F32, tag="ball", name="ball")
            bk_t = ball_t[:, 0:NCH]
            bq_t = ball_t[:, NCH:2 * NCH]
            bv_t = consts.tile([128, 1], F32, tag="bv", name="bv")
            id_t = consts.tile([128, 128], F32R, tag="ident", name="ident")
            ones_b = consts.tile([128, 1], BF16, tag="ones_b", name="ones_b")
            nc.vector.memset(ones_b, 1.0)

            def load_rest_consts():
                nc.scalar.dma_start(out=wall_t[:, :, C:], in_=wallv[:, :, C:])
                nc.sync.dma_start(out=ball_t, in_=ball_d)
                nc.sync.dma_start(out=bv_t, in_=_ap(bv_d, 0, [[0, 128], [1, 1]]))

            def load_ident():
                nc.scalar.dma_start(out=id_t, in_=id_d)

            def warmup():
                # ones_b comes from a memset (no DMA dependency), so these
                # N=1 matmuls start immediately and keep the PE busy through
                # the input-load window -> HAM unthrottles before the first
                # projection matmul.
                pw = small.tile([128, HW], F32, tag="small", name="warm")
                for _ in range(30):
                    nc.tensor.matmul(pw[0:1, 0:1], ones_b, ones_b,
                                     start=True, stop=True)

            # ---------------- per-b state ----------------
            xs, kns, qts, qns, sqs, v1s = {}, {}, {}, {}, {}, {}
            rqts, rkts, rq_bcs, V_bcs, pnds, pssqs = {}, {}, {}, {}, {}, {}
            d_ssqQ, d_ssqK, d_v, d_nd, d_V = {}, {}, {}, {}, {}

            def load_x(b):
                xs[b] = sbuf.tile([128, NCH, HW], F32R, tag="x", name="x")
                xv = x_d[b].rearrange("(kc p) m -> p kc m", p=128)
                engs = [nc.scalar, nc.sync]
                for kc in range(NCH):
                    engs[kc].dma_start(out=xs[b][:, kc, :], in_=xv[:, kc, :])

            def mm_proj(psum, w_t, mc, b):
                for kc in range(NCH):
                    for nt in range(2):
                        nc.tensor.matmul(
                            psum[:, nt * 512:(nt + 1) * 512],
                            w_t[:, kc, mc * 128:(mc + 1) * 128],
                            xs[b][:, kc, nt * 512:(nt + 1) * 512],
                            start=(kc == 0), stop=(kc == NCH - 1))

            def mm_sumsq(pssq, sq, ti):
                # accumulate over kc; Q -> row 0, K -> row 32
                for nt in range(2):
                    for kc in range(NCH):
                        nc.tensor.matmul(
                            pssq[32 * ti:32 * ti + 1, nt * 512:(nt + 1) * 512],
                            ones_b,
                            sq[:, ti, kc, nt * 512:(nt + 1) * 512],
                            start=(kc == 0), stop=(kc == NCH - 1))

            def projQ(b):
                qts[b] = sbuf.tile([128, NCH, HW], F32, tag="qt", name="qt")
                sq = sbuf.tile([128, 2, NCH, HW], BF16, tag="sq", name="sq")
                sqs[b] = sq
                for mc in range(NCH):
                    pp = big.tile([128, HW], F32, tag="big", name="big")
                    mm_proj(pp, wq_t, mc, b)
                    nc.vector.tensor_scalar_add(
                        qts[b][:, mc, :], pp, bq_t[:, mc:mc + 1])
                    nc.scalar.activation(
                        sq[:, 0, mc, :], pp, AF.Square, bias=bq_t[:, mc:mc + 1])
                pssq = small.tile([128, HW], F32, tag="small", name="small")
                pssqs[b] = pssq
                mm_sumsq(pssq, sq, 0)
                s_sq = sbuf.tile([1, HW], F32, tag="s_sqQ", name="s_sqQ")
                nc.vector.tensor_copy(s_sq, pssq[0:1, :])
                d_ssqQ[b] = dramp.tile([1, HW], F32, tag="d_ssqQ", name="d_ssqQ")
                nc.sync.dma_start(out=d_ssqQ[b], in_=s_sq)

            def rqchain(b):
                # ssqQ[p, j] = d_ssqQ[0, p*8+j] -> rq = exp(-0.5 ln(.))
                t = sbuf.tile([128, NNJ], F32, tag="rqt", name="rqt")
                nc.sync.dma_start(
                    out=t, in_=_ap(d_ssqQ[b], 0, [[NNJ, 128], [1, NNJ]]))
                nc.scalar.activation(t, t, AF.Ln)
                nc.scalar.activation(t, t, AF.Exp, scale=-0.5)
                rqts[b] = t
                d_rq = dramp.tile([1, HW], F32, tag="d_rq", name="d_rq")
                nc.sync.dma_start(
                    out=_ap(d_rq, 0, [[NNJ, 128], [1, NNJ]]), in_=t)
                rq_bc = sbuf.tile([128, HW], F32, tag="rq_bc", name="rq_bc")
                nc.sync.dma_start(
                    out=rq_bc, in_=_ap(d_rq, 0, [[0, 128], [1, HW]]))
                rq_bcs[b] = rq_bc

            def projKV(b):
                kns[b] = sbuf.tile([128, NCH, HW], BF16, tag="kn", name="kn")
                sq = sqs[b]
                for mc in range(NCH):
                    pp = big.tile([128, HW], F32, tag="big", name="big")
                    mm_proj(pp, wk_t, mc, b)
                    nc.vector.tensor_scalar_add(
                        kns[b][:, mc, :], pp, bk_t[:, mc:mc + 1])
                    nc.scalar.activation(
                        sq[:, 1, mc, :], pp, AF.Square, bias=bk_t[:, mc:mc + 1])
                mm_sumsq(pssqs[b], sq, 1)
                s_sq = sbuf.tile([33, HW], F32, tag="s_sqK", name="s_sqK")
                nc.vector.tensor_copy(s_sq[32:33, :], pssqs[b][32:33, :])
                d_ssqK[b] = dramp.tile([1, HW], F32, tag="d_ssqK", name="d_ssqK")
                nc.gpsimd.dma_start(out=d_ssqK[b], in_=s_sq[32:33, :])
                # v projection (M=1)
                pv = small.tile([128, HW], F32, tag="small", name="small")
                for kc in range(NCH):
                    for nt in range(2):
                        nc.tensor.matmul(
                            pv[0:1, nt * 512:(nt + 1) * 512],
                            wv_t[:, kc, 0:1],
                            xs[b][:, kc, nt * 512:(nt + 1) * 512],
                            start=(kc == 0), stop=(kc == NCH - 1))
                sv = sbuf.tile([1, HW], F32, tag="sv", name="sv")
                nc.vector.tensor_copy(sv, pv[0:1, :])
                d_v[b] = dramp.tile([1, HW], F32, tag="d_v", name="d_v")
                nc.gpsimd.dma_start(out=d_v[b], in_=sv)

            def qnorm(b):
                qns[b] = sbuf.tile([128, NCH, HW], BF16, tag="qn", name="qn")
                for mc in range(NCH):
                    nc.vector.tensor_mul(
                        qns[b][:, mc, :], qts[b][:, mc, :], rq_bcs[b])

            def rkchain(b):
                t = sbuf.tile([128, NNJ], F32, tag="rkt", name="rkt")
                nc.sync.dma_start(
                    out=t, in_=_ap(d_ssqK[b], 0, [[NNJ, 128], [1, NNJ]]))
                nc.scalar.activation(t, t, AF.Ln)
                nc.scalar.activation(t, t, AF.Exp, scale=-0.5)
                rkts[b] = t

            def vprep(b):
                v_sb = sbuf.tile([128, NNJ], F32, tag="v_sb", name="v_sb")
                nc.sync.dma_start(
                    out=v_sb, in_=_ap(d_v[b], 0, [[NNJ, 128], [1, NNJ]]))
                v1f = sbuf.tile([128, NNJ, 2], F32, tag="v1f", name="v1f")
                nc.vector.memset(v1f, 1.0)
                nc.vector.tensor_scalar_add(v1f[:, :, 0], v_sb, bv_t[:, 0:1])
                v1r = sbuf.tile([128, NNJ, 2], F32R, tag="v1r", name="v1r")
                nc.vector.tensor_copy(v1r, v1f)
                v1s[b] = v1r

            def gram(b):
                # row-block j of E holds rows n = p*8 + j
                knv = kns[b].rearrange("p kc (q j) -> p kc q j", j=NNJ)
                pnd = small.tile([128, HW], F32, tag="small", name="small")
                pnds[b] = pnd
                pgs, es = [None] * NNJ, [None] * NNJ

                def gram_chunk(nj):
                    pg = big.tile([128, HW], F32, tag="big", name="big")
                    pgs[nj] = pg
                    for kc in range(NCH):
                        for nt in range(2):
                            nc.tensor.matmul(
                                pg[:, nt * 512:(nt + 1) * 512],
                                knv[:, kc, :, nj],
                                qns[b][:, kc, nt * 512:(nt + 1) * 512],
                                start=(kc == 0), stop=(kc == NCH - 1))

                def exp_chunk(nj):
                    e = epool.tile([128, HW], F32R, tag="e", name="e")
                    es[nj] = e
                    nc.scalar.activation(
                        e, pgs[nj], AF.Exp, scale=rkts[b][:, nj:nj + 1])

                def numer_chunk(nj):
                    for nt in range(2):
                        nc.tensor.matmul(
                            pnd[0:2, nt * 512:(nt + 1) * 512],
                            v1s[b][:, nj, :],
                            es[nj][:, nt * 512:(nt + 1) * 512],
                            start=(nj == 0), stop=(nj == NNJ - 1))

                gram_chunk(0)
                exp_chunk(0)
                for nj in range(1, NNJ):
                    gram_chunk(nj)
                    exp_chunk(nj)
                    numer_chunk(nj - 1)
                numer_chunk(NNJ - 1)

            def vcalc(b):
                s_nd = sbuf.tile([2, HW], F32, tag="s_nd", name="s_nd")
                nc.scalar.copy(s_nd, pnds[b][0:2, :])
                d_nd[b] = dramp.tile([2, HW], F32, tag="d_nd", name="d_nd")
                nc.sync.dma_start(out=d_nd[b], in_=s_nd)
                # nd[p, r, j] = d_nd[r, p*8 + j]
                nd = sbuf.tile([128, 2, NNJ], F32, tag="nd", name="nd")
                nc.scalar.dma_start(
                    out=nd, in_=_ap(d_nd[b], 0, [[NNJ, 128], [HW, 2], [1, NNJ]]))
                rden = sbuf.tile([128, NNJ], F32, tag="rden", name="rden")
                nc.vector.reciprocal(rden, nd[:, 1, :])
                Vt = sbuf.tile([128, NNJ], F32, tag="Vt", name="Vt")
                nc.vector.tensor_mul(Vt, nd[:, 0, :], rden)
                d_V[b] = dramp.tile([1, HW], F32, tag="d_V", name="d_V")
                nc.sync.dma_start(
                    out=_ap(d_V[b], 0, [[NNJ, 128], [1, NNJ]]), in_=Vt)
                V_bc = sbuf.tile([128, HW], F32, tag="V_bc", name="V_bc")
                for h, eng in [(0, nc.scalar), (1, nc.sync)]:
                    eng.dma_start(
                        out=V_bc[:, h * 512:(h + 1) * 512],
                        in_=_ap(d_V[b], h * 512, [[0, 128], [1, 512]]))
                V_bcs[b] = V_bc

            def conv(b):
                x_f = xs[b].bitcast(F32)
                for mc in range(NCH):
                    p_sb = sbuf.tile([128, HW], F32, tag="p_sb", name="p_sb")
                    nc.vector.tensor_mul(p_sb, x_f[:, mc, :], V_bcs[b])
                    pv3 = p_sb.rearrange("p (h w) -> p h w", h=H)
                    t1 = sbuf.tile([128, H, 31], F32, tag="t1", name="t1")
                    nc.vector.tensor_add(t1, pv3[:, :, 0:31], pv3[:, :, 1:32])
                    s1 = sbuf.tile([128, H, CW], F32R, tag="s1", name="s1")
                    nc.vector.tensor_add(s1, t1[:, :, 0:CW], pv3[:, :, 2:32])
                    s1f = s1.rearrange("p h w -> p (h w)")
                    pout = big.tile([128, HW], F32, tag="big", name="big")
                    for dh in range(3):
                        for lo, sz in [(0, 512), (512, NF - 512)]:
                            nc.tensor.matmul(
                                pout[:, lo:lo + sz],
                                id_t,
                                s1f[:, dh * CW + lo: dh * CW + lo + sz],
                                start=(dh == 0), stop=(dh == 2))
                    s_out = sbuf.tile([128, NF], F32, tag="s_out", name="s_out")
                    nc.scalar.copy(s_out, pout[:, 0:NF])
                    nc.gpsimd.dma_start(
                        out=out_d[b, mc * 128:(mc + 1) * 128, :],
                        in_=s_out)

            # ---------------- emission schedule ----------------
            # rq chains launch right after each Q-projection so the DRAM
            # round trips hide behind the remaining projection matmuls.
            load_x(0)
            load_rest_consts()
            load_x(1)
            load_ident()
            warmup()
            for b in range(BL):
                projQ(b)
                rqchain(b)
                projKV(b)
                qnorm(b)
                rkchain(b)
                vprep(b)
            gram(0)
            vcalc(0)
            gram(1)
            conv(0)
            vcalc(1)
            conv(1)

    nc.compile()
    return nc


_CACHE = {}


def _get_program():
    if "nc" not in _CACHE:
        _CACHE["nc"] = build_program()
    return _CACHE["nc"]


def make_in_maps(batch, key_w, key_b, query_w, query_b, value_w, value_b):
    wall = np.zeros((C, 2 * C + 2), np.float32)
    wall[:, 0:C] = query_w.T
    wall[:, C:2 * C] = key_w.T
    wall[:, 2 * C] = value_w[0]
    ball = np.zeros((128, 2 * NCH), np.float32)
    ball[:, 0:NCH] = key_b.reshape(NCH, 128).T
    ball[:, NCH:2 * NCH] = query_b.reshape(NCH, 128).T
    bv = np.zeros((1, 2), np.float32)
    bv[0, 0] = value_b[0]
    ident = np.eye(128, dtype=np.float32)
    in_maps = []
    for i in range(NCORES):
        xb = batch[i * BL:(i + 1) * BL].reshape(BL, C, HW)
        in_maps.append({
            "x": np.ascontiguousarray(xb),
            "wall": wall, "ball": ball, "bv": bv,
            "ident": ident,
        })
    return in_maps


def kernel(batch, key_w, key_b, query_w, query_b, value_w, value_b,
           local_indices=None, **_ignored):
    batch = np.ascontiguousarray(np.asarray(batch, np.float32))
    args = [np.asarray(a, np.float32) for a in
            (key_w, key_b, query_w, query_b, value_w, value_b)]
    nc = _get_program()
    in_maps = make_in_maps(batch, *args)
    res = run_bass_kernel_spmd(nc, in_maps, list(range(NCORES)))
    outs = [np.asarray(r["out"], np.float32) for r in res.results]
    return np.concatenate(outs, axis=0).reshape(B, C, CH, CW)

